# revision 17
# baseline (speedup 1.0000x reference)
"""Causal depthwise conv1d (B=8, C=1024, T=8192, K=4, dil=1) on 8 trn2 cores.

Sharding: batch-parallel — core j handles x[j] (1024, 8192), communication-free.

Memory-bound problem: the fp32 version sat at the HBM roofline (64 MiB/core),
so x and y travel as fp16 (host converts; gate is rel<2e-2, fp16 costs ~7e-4),
halving traffic to 32 MiB/core.

Phase-interleaved formulation (host-prepared layout): each channel occupies
4 SBUF partitions, one per time-phase (t mod 4):
    x_il[4c+psi, m] = xpad[c, 4m+psi]   (xpad = x left-padded with 4 zeros)
Time shifts become partition shifts, so the whole K=4 depthwise conv folds
into TWO matmuls with banded block-diagonal stationary matrices:
    psum[4c+phi, n] = L0^T @ x_il[:, n+1] + L1^T @ x_il[:, n]
    L0[4c+psi, 4c+phi] = w[c, psi-phi+3]  (0 <= j <= 3)
    L1[4c+psi, 4c+phi] = w[c, psi-phi-1]  (0 <= j <= 2)
All 4 taps ride the PE (2 x 226ns per 512-col group, ~17% busy); the only
elementwise work left is the PSUM evict + bias, alternated between ACT and
DVE (~11% each). The kernel is purely DMA-bound: loads on the SP HWDGE
ring, stores on the POOL ring (cheap 25ns seq config, engine otherwise
idle). ot-slot WAR edges for store DMAs are added via add_dep_helper.
"""
import numpy as np

import concourse.bacc as bacc
import concourse.mybir as mybir
from concourse.tile import TileContext
from concourse.tile import add_dep_helper
from concourse import bass_utils

B, C, T, K = 8, 1024, 8192, 4
P = 128               # SBUF partitions
PH = 4                # time phases (interleave factor)
CH = P // PH          # 32 channels per row-block
RBLK = C // CH        # 8 row-blocks per core
U = T // PH           # 2048 phase-major columns
CHUNK_U = U           # one load/store tile per row-block
GRP = 512             # psum group width (one bank)
NGRP = CHUNK_U // GRP  # 4 groups per row-block
IOBUFS = 4            # xt pool bufs
OTBUFS = 4            # ot pool bufs (slot-reuse distance for the WAR dep)

_cached = {}


def _build():
    nc = bacc.Bacc("TRN2", target_bir_lowering=False, debug=False)
    f32 = mybir.dt.float32
    f16 = mybir.dt.float16

    x_d = nc.dram_tensor("x", [PH * C, U + 1], f16, kind="ExternalInput")
    wd_d = nc.dram_tensor("wd", [P, RBLK * 2 * P], f16, kind="ExternalInput")
    b_d = nc.dram_tensor("bv", [P, RBLK], f32, kind="ExternalInput")
    y_d = nc.dram_tensor("y", [PH * C, U], f16, kind="ExternalOutput")

    with TileContext(nc) as tc:
        with (
            tc.tile_pool(name="const", bufs=1) as cpool,
            tc.tile_pool(name="io", bufs=IOBUFS) as pool,
            tc.tile_pool(name="ox", bufs=OTBUFS) as opool,
            tc.tile_pool(name="psum", bufs=6, space="PSUM") as psum_pool,
        ):
            wt = cpool.tile([P, RBLK * 2 * P], f16)
            # weight stream rides the ACT ring (no other DMA traffic there;
            # loads own SP, stores own POOL), in 4 pieces so rb=0's matmuls
            # unblock after the first 512KB
            WPC = RBLK // 4
            for wp in range(4):
                nc.scalar.dma_start(
                    out=wt[:, wp * WPC * 2 * P:(wp + 1) * WPC * 2 * P],
                    in_=wd_d.ap()[:, wp * WPC * 2 * P:(wp + 1) * WPC * 2 * P])
            bt = cpool.tile([P, RBLK], f32)
            nc.sync.dma_start(out=bt, in_=b_d.ap())

            store_insts = []
            ngl = 0  # global group counter (evict engine alternation)
            for rb in range(RBLK):
                rows = slice(rb * P, (rb + 1) * P)
                xt = pool.tile([P, CHUNK_U + 1], f16, tag="xt")
                if rb == 0:
                    # split the first load so group g starts as soon as
                    # its piece lands
                    for g4 in range(NGRP):
                        a = 0 if g4 == 0 else g4 * GRP + 1
                        e = (g4 + 1) * GRP + 1
                        nc.sync.dma_start(
                            out=xt[:, a:e],
                            in_=x_d.ap()[rows, a:e])
                else:
                    nc.sync.dma_start(out=xt, in_=x_d.ap()[rows, :])

                ot = opool.tile([P, CHUNK_U], f16, tag="ot")
                for g in range(NGRP):
                    a = g * GRP
                    ps = psum_pool.tile([P, GRP], f32)
                    nc.tensor.matmul(
                        ps, wt[:, (rb * 2) * P:(rb * 2 + 1) * P],
                        xt[:, a + 1:a + GRP + 1],
                        start=True, stop=False)
                    nc.tensor.matmul(
                        ps, wt[:, (rb * 2 + 1) * P:(rb * 2 + 2) * P],
                        xt[:, a:a + GRP],
                        start=False, stop=True)
                    # PSUM evict + bias, alternating ACT / DVE
                    if ngl % 2 == 0:
                        ev = nc.scalar.activation(
                            ot[:, a:a + GRP], ps,
                            mybir.ActivationFunctionType.Identity,
                            bias=bt[:, rb:rb + 1], scale=1.0)
                    else:
                        ev = nc.vector.tensor_scalar_add(
                            out=ot[:, a:a + GRP], in0=ps,
                            scalar1=bt[:, rb:rb + 1])
                    ngl += 1
                    if g == 0 and rb >= OTBUFS:
                        add_dep_helper(
                            ev.ins, store_insts[rb - OTBUFS].ins,
                            reason="ot slot reuse waits for store DMA")
                    if rb == RBLK - 1:
                        # final row-block: store per group so the tail
                        # drains as soon as each eviction lands
                        st = nc.gpsimd.dma_start(
                            out=y_d.ap()[rows, a:a + GRP],
                            in_=ot[:, a:a + GRP])
                if rb < RBLK - 1:
                    st = nc.gpsimd.dma_start(
                        out=y_d.ap()[rows, :], in_=ot)
                store_insts.append(st)
    nc.compile()
    return nc


def _host_weights(w, b):
    # banded block-diagonal stationary matrices, one (L0, L1) pair per
    # row-block: L0 carries taps with time-shift 0, L1 shift -1 (see module
    # docstring)
    wd = np.zeros((P, RBLK * 2 * P), dtype=np.float16)
    for rb in range(RBLK):
        for c in range(CH):
            for phi in range(PH):
                for psi in range(PH):
                    j0 = psi - phi + 3
                    if 0 <= j0 <= 3:
                        wd[PH * c + psi, (rb * 2) * P + PH * c + phi] = \
                            np.float16(w[CH * rb + c, 0, j0])
                    j1 = psi - phi - 1
                    if 0 <= j1 <= 2:
                        wd[PH * c + psi, (rb * 2 + 1) * P + PH * c + phi] = \
                            np.float16(w[CH * rb + c, 0, j1])
    bv = np.empty((P, RBLK), dtype=np.float32)
    for rb in range(RBLK):
        bv[:, rb] = np.repeat(b[CH * rb:CH * (rb + 1)], PH)
    return wd, bv


def _host_x(xj):
    # fp16 quantize + causal zero pad + phase-interleave:
    # x_il[4c+psi, m] = xpad[c, 4m+psi]
    xpad = np.zeros((C, T + PH), dtype=np.float16)
    xpad[:, PH:] = xj
    return np.ascontiguousarray(
        xpad.reshape(C, U + 1, PH).transpose(0, 2, 1).reshape(PH * C, U + 1))


def _host_y(y_il):
    # de-interleave: y[c, 4u+phi] = y_il[4c+phi, u]
    return np.ascontiguousarray(
        y_il.astype(np.float32).reshape(C, PH, U).transpose(0, 2, 1)
        .reshape(C, T))


def kernel(x, w, b):
    x = np.asarray(x, dtype=np.float32)
    w = np.asarray(w, dtype=np.float32)
    b = np.asarray(b, dtype=np.float32)

    if "nc" not in _cached:
        _cached["nc"] = _build()
    nc = _cached["nc"]

    wd, bv = _host_weights(w, b)
    in_maps = [
        {"x": _host_x(x[j]), "wd": wd, "bv": bv}
        for j in range(B)
    ]
    res = bass_utils.run_bass_kernel_spmd(nc, in_maps, core_ids=list(range(B)))
    return np.stack([_host_y(r["y"]) for r in res.results], axis=0)


# revision 18
# speedup vs baseline: 1.0198x; 1.0198x over previous
"""Causal depthwise conv1d (B=8, C=1024, T=8192, K=4, dil=1) on 8 trn2 cores.

Sharding: batch-parallel — core j handles x[j] (1024, 8192), communication-free.

Memory-bound problem: the fp32 version sat at the HBM roofline (64 MiB/core),
so x and y travel as fp16 (host converts; gate is rel<2e-2, fp16 costs ~7e-4),
halving traffic to 32 MiB/core.

Phase-interleaved formulation (host-prepared layout): each channel occupies
4 SBUF partitions, one per time-phase (t mod 4):
    x_il[4c+psi, m] = xpad[c, 4m+psi]   (xpad = x left-padded with 4 zeros)
Time shifts become partition shifts, so the whole K=4 depthwise conv folds
into TWO matmuls with banded block-diagonal stationary matrices:
    psum[4c+phi, n] = L0^T @ x_il[:, n+1] + L1^T @ x_il[:, n]
    L0[4c+psi, 4c+phi] = w[c, psi-phi+3]  (0 <= j <= 3)
    L1[4c+psi, 4c+phi] = w[c, psi-phi-1]  (0 <= j <= 2)
All 4 taps ride the PE (2 x 226ns per 512-col group, ~17% busy); the only
elementwise work left is the PSUM evict + bias, alternated between ACT and
DVE (~11% each). The kernel is purely DMA-bound: loads on the SP HWDGE
ring, stores on the POOL ring (cheap 25ns seq config, engine otherwise
idle). ot-slot WAR edges for store DMAs are added via add_dep_helper.
"""
import numpy as np

import concourse.bacc as bacc
import concourse.mybir as mybir
from concourse.tile import TileContext
from concourse.tile import add_dep_helper
from concourse import bass_utils

B, C, T, K = 8, 1024, 8192, 4
P = 128               # SBUF partitions
PH = 4                # time phases (interleave factor)
CH = P // PH          # 32 channels per row-block
RBLK = C // CH        # 8 row-blocks per core
U = T // PH           # 2048 phase-major columns
CHUNK_U = U           # one load/store tile per row-block
GRP = 512             # psum group width (one bank)
NGRP = CHUNK_U // GRP  # 4 groups per row-block
IOBUFS = 8            # xt pool bufs
OTBUFS = 8            # ot pool bufs (slot-reuse distance for the WAR dep)

_cached = {}


def _build():
    nc = bacc.Bacc("TRN2", target_bir_lowering=False, debug=False)
    f32 = mybir.dt.float32
    f16 = mybir.dt.float16

    x_d = nc.dram_tensor("x", [PH * C, U + 1], f16, kind="ExternalInput")
    wd_d = nc.dram_tensor("wd", [P, RBLK * 2 * P], f16, kind="ExternalInput")
    b_d = nc.dram_tensor("bv", [P, RBLK], f32, kind="ExternalInput")
    y_d = nc.dram_tensor("y", [PH * C, U], f16, kind="ExternalOutput")

    with TileContext(nc) as tc:
        with (
            tc.tile_pool(name="const", bufs=1) as cpool,
            tc.tile_pool(name="io", bufs=IOBUFS) as pool,
            tc.tile_pool(name="ox", bufs=OTBUFS) as opool,
            tc.tile_pool(name="psum", bufs=8, space="PSUM") as psum_pool,
        ):
            wt = cpool.tile([P, RBLK * 2 * P], f16)
            # weight stream rides the ACT ring (no other DMA traffic there;
            # loads own SP, stores own POOL), in 4 pieces so rb=0's matmuls
            # unblock after the first 512KB
            WPC = RBLK // 4
            for wp in range(4):
                nc.scalar.dma_start(
                    out=wt[:, wp * WPC * 2 * P:(wp + 1) * WPC * 2 * P],
                    in_=wd_d.ap()[:, wp * WPC * 2 * P:(wp + 1) * WPC * 2 * P])
            bt = cpool.tile([P, RBLK], f32)
            nc.sync.dma_start(out=bt, in_=b_d.ap())

            store_insts = []
            ngl = 0  # global group counter (evict engine alternation)
            for rb in range(RBLK):
                rows = slice(rb * P, (rb + 1) * P)
                xt = pool.tile([P, CHUNK_U + 1], f16, tag="xt")
                if rb == 0:
                    # split the first load so group g starts as soon as
                    # its piece lands
                    for g4 in range(NGRP):
                        a = 0 if g4 == 0 else g4 * GRP + 1
                        e = (g4 + 1) * GRP + 1
                        nc.sync.dma_start(
                            out=xt[:, a:e],
                            in_=x_d.ap()[rows, a:e])
                else:
                    # alternate load ring SP / ACT to use both DGE paths
                    ldq = nc.sync if rb % 2 == 0 else nc.scalar
                    ldq.dma_start(out=xt, in_=x_d.ap()[rows, :])

                ot = opool.tile([P, CHUNK_U], f16, tag="ot")
                for g in range(NGRP):
                    a = g * GRP
                    ps = psum_pool.tile([P, GRP], f32)
                    nc.tensor.matmul(
                        ps, wt[:, (rb * 2) * P:(rb * 2 + 1) * P],
                        xt[:, a + 1:a + GRP + 1],
                        start=True, stop=False)
                    nc.tensor.matmul(
                        ps, wt[:, (rb * 2 + 1) * P:(rb * 2 + 2) * P],
                        xt[:, a:a + GRP],
                        start=False, stop=True)
                    # PSUM evict + bias, alternating ACT / DVE
                    if ngl % 2 == 0:
                        ev = nc.scalar.activation(
                            ot[:, a:a + GRP], ps,
                            mybir.ActivationFunctionType.Identity,
                            bias=bt[:, rb:rb + 1], scale=1.0)
                    else:
                        ev = nc.vector.tensor_scalar_add(
                            out=ot[:, a:a + GRP], in0=ps,
                            scalar1=bt[:, rb:rb + 1])
                    ngl += 1
                    if g == 0 and rb >= OTBUFS:
                        add_dep_helper(
                            ev.ins, store_insts[rb - OTBUFS].ins,
                            reason="ot slot reuse waits for store DMA")
                    if rb == RBLK - 1:
                        # final row-block: store per group so the tail
                        # drains as soon as each eviction lands
                        st = nc.gpsimd.dma_start(
                            out=y_d.ap()[rows, a:a + GRP],
                            in_=ot[:, a:a + GRP])
                if rb < RBLK - 1:
                    st = nc.gpsimd.dma_start(
                        out=y_d.ap()[rows, :], in_=ot)
                store_insts.append(st)
    nc.compile()
    return nc


def _host_weights(w, b):
    # banded block-diagonal stationary matrices, one (L0, L1) pair per
    # row-block: L0 carries taps with time-shift 0, L1 shift -1 (see module
    # docstring)
    wd = np.zeros((P, RBLK * 2 * P), dtype=np.float16)
    for rb in range(RBLK):
        for c in range(CH):
            for phi in range(PH):
                for psi in range(PH):
                    j0 = psi - phi + 3
                    if 0 <= j0 <= 3:
                        wd[PH * c + psi, (rb * 2) * P + PH * c + phi] = \
                            np.float16(w[CH * rb + c, 0, j0])
                    j1 = psi - phi - 1
                    if 0 <= j1 <= 2:
                        wd[PH * c + psi, (rb * 2 + 1) * P + PH * c + phi] = \
                            np.float16(w[CH * rb + c, 0, j1])
    bv = np.empty((P, RBLK), dtype=np.float32)
    for rb in range(RBLK):
        bv[:, rb] = np.repeat(b[CH * rb:CH * (rb + 1)], PH)
    return wd, bv


def _host_x(xj):
    # fp16 quantize + causal zero pad + phase-interleave:
    # x_il[4c+psi, m] = xpad[c, 4m+psi]
    xpad = np.zeros((C, T + PH), dtype=np.float16)
    xpad[:, PH:] = xj
    return np.ascontiguousarray(
        xpad.reshape(C, U + 1, PH).transpose(0, 2, 1).reshape(PH * C, U + 1))


def _host_y(y_il):
    # de-interleave: y[c, 4u+phi] = y_il[4c+phi, u]
    return np.ascontiguousarray(
        y_il.astype(np.float32).reshape(C, PH, U).transpose(0, 2, 1)
        .reshape(C, T))


def kernel(x, w, b):
    x = np.asarray(x, dtype=np.float32)
    w = np.asarray(w, dtype=np.float32)
    b = np.asarray(b, dtype=np.float32)

    if "nc" not in _cached:
        _cached["nc"] = _build()
    nc = _cached["nc"]

    wd, bv = _host_weights(w, b)
    in_maps = [
        {"x": _host_x(x[j]), "wd": wd, "bv": bv}
        for j in range(B)
    ]
    res = bass_utils.run_bass_kernel_spmd(nc, in_maps, core_ids=list(range(B)))
    return np.stack([_host_y(r["y"]) for r in res.results], axis=0)


# revision 19
# speedup vs baseline: 1.1296x; 1.1076x over previous
"""Causal depthwise conv1d (B=8, C=1024, T=8192, K=4, dil=1) on 8 trn2 cores.

Sharding: batch-parallel — core j handles x[j] (1024, 8192), communication-free.

Memory-bound problem: the fp32 version sat at the HBM roofline (64 MiB/core),
so x and y travel as fp16 (host converts; gate is rel<2e-2, fp16 costs ~7e-4),
halving traffic to 32 MiB/core.

Phase-interleaved formulation (host-prepared layout): each channel occupies
4 SBUF partitions, one per time-phase (t mod 4):
    x_il[4c+psi, m] = xpad[c, 4m+psi]   (xpad = x left-padded with 4 zeros)
Time shifts become partition shifts, so the whole K=4 depthwise conv folds
into TWO matmuls with banded block-diagonal stationary matrices:
    psum[4c+phi, n] = L0^T @ x_il[:, n+1] + L1^T @ x_il[:, n]
    L0[4c+psi, 4c+phi] = w[c, psi-phi+3]  (0 <= j <= 3)
    L1[4c+psi, 4c+phi] = w[c, psi-phi-1]  (0 <= j <= 2)
All 4 taps ride the PE (2 x 226ns per 512-col group, ~17% busy); the only
elementwise work left is the PSUM evict + bias, alternated between ACT and
DVE (~11% each). The kernel is purely DMA-bound: loads on the SP HWDGE
ring, stores on the POOL ring (cheap 25ns seq config, engine otherwise
idle). ot-slot WAR edges for store DMAs are added via add_dep_helper.
"""
import numpy as np

import concourse.bacc as bacc
import concourse.mybir as mybir
from concourse.tile import TileContext
from concourse.tile import add_dep_helper
from concourse import bass_utils

B, C, T, K = 8, 1024, 8192, 4
P = 128               # SBUF partitions
PH = 4                # time phases (interleave factor)
CH = P // PH          # 32 channels per row-block
RBLK = C // CH        # 8 row-blocks per core
U = T // PH           # 2048 phase-major columns
CHUNK_U = U           # one load/store tile per row-block
GRP = 512             # psum group width (one bank)
NGRP = CHUNK_U // GRP  # 4 groups per row-block
IOBUFS = 8            # xt pool bufs
OTBUFS = 8            # ot pool bufs (slot-reuse distance for the WAR dep)

_cached = {}


def _build():
    nc = bacc.Bacc("TRN2", target_bir_lowering=False, debug=False)
    f32 = mybir.dt.float32
    f16 = mybir.dt.float16

    x_d = nc.dram_tensor("x", [PH * C, U + 1], f16, kind="ExternalInput")
    wd_d = nc.dram_tensor("wd", [P, RBLK * 2 * P], f16, kind="ExternalInput")
    b_d = nc.dram_tensor("bv", [P, RBLK], f32, kind="ExternalInput")
    y_d = nc.dram_tensor("y", [PH * C, U], f16, kind="ExternalOutput")

    with TileContext(nc) as tc:
        with (
            tc.tile_pool(name="const", bufs=1) as cpool,
            tc.tile_pool(name="io", bufs=IOBUFS) as pool,
            tc.tile_pool(name="ox", bufs=OTBUFS) as opool,
            tc.tile_pool(name="psum", bufs=8, space="PSUM") as psum_pool,
        ):
            wt = cpool.tile([P, RBLK * 2 * P], f16)
            # weight stream rides the POOL ring (SWDGE, but small and at
            # kernel start when stores haven't begun), in 4 pieces so rb=0's
            # matmuls unblock after the first 512KB
            WPC = RBLK // 4
            for wp in range(4):
                nc.gpsimd.dma_start(
                    out=wt[:, wp * WPC * 2 * P:(wp + 1) * WPC * 2 * P],
                    in_=wd_d.ap()[:, wp * WPC * 2 * P:(wp + 1) * WPC * 2 * P])
            bt = cpool.tile([P, RBLK], f32)
            nc.sync.dma_start(out=bt, in_=b_d.ap())

            store_insts = []
            ngl = 0  # global group counter (evict engine alternation)
            for rb in range(RBLK):
                rows = slice(rb * P, (rb + 1) * P)
                xt = pool.tile([P, CHUNK_U + 1], f16, tag="xt")
                if rb == 0:
                    # split the first load so group g starts as soon as
                    # its piece lands
                    for g4 in range(NGRP):
                        a = 0 if g4 == 0 else g4 * GRP + 1
                        e = (g4 + 1) * GRP + 1
                        nc.sync.dma_start(
                            out=xt[:, a:e],
                            in_=x_d.ap()[rows, a:e])
                else:
                    nc.sync.dma_start(out=xt, in_=x_d.ap()[rows, :])

                ot = opool.tile([P, CHUNK_U], f16, tag="ot")
                for g in range(NGRP):
                    a = g * GRP
                    ps = psum_pool.tile([P, GRP], f32)
                    nc.tensor.matmul(
                        ps, wt[:, (rb * 2) * P:(rb * 2 + 1) * P],
                        xt[:, a + 1:a + GRP + 1],
                        start=True, stop=False)
                    nc.tensor.matmul(
                        ps, wt[:, (rb * 2 + 1) * P:(rb * 2 + 2) * P],
                        xt[:, a:a + GRP],
                        start=False, stop=True)
                    # PSUM evict + bias, alternating ACT / DVE
                    if ngl % 2 == 0:
                        ev = nc.scalar.activation(
                            ot[:, a:a + GRP], ps,
                            mybir.ActivationFunctionType.Identity,
                            bias=bt[:, rb:rb + 1], scale=1.0)
                    else:
                        ev = nc.vector.tensor_scalar_add(
                            out=ot[:, a:a + GRP], in0=ps,
                            scalar1=bt[:, rb:rb + 1])
                    ngl += 1
                    if g == 0 and rb >= OTBUFS:
                        add_dep_helper(
                            ev.ins, store_insts[rb - OTBUFS].ins,
                            reason="ot slot reuse waits for store DMA")
                    if rb == RBLK - 1:
                        # final row-block: store per group so the tail
                        # drains as soon as each eviction lands
                        st = nc.scalar.dma_start(
                            out=y_d.ap()[rows, a:a + GRP],
                            in_=ot[:, a:a + GRP])
                if rb < RBLK - 1:
                    # stores ride the ACT HWDGE ring (gpsimd DMA is SWDGE —
                    # 16MB of software packet generation was the bottleneck)
                    st = nc.scalar.dma_start(
                        out=y_d.ap()[rows, :], in_=ot)
                store_insts.append(st)
    nc.compile()
    return nc


def _host_weights(w, b):
    # banded block-diagonal stationary matrices, one (L0, L1) pair per
    # row-block: L0 carries taps with time-shift 0, L1 shift -1 (see module
    # docstring)
    wd = np.zeros((P, RBLK * 2 * P), dtype=np.float16)
    for rb in range(RBLK):
        for c in range(CH):
            for phi in range(PH):
                for psi in range(PH):
                    j0 = psi - phi + 3
                    if 0 <= j0 <= 3:
                        wd[PH * c + psi, (rb * 2) * P + PH * c + phi] = \
                            np.float16(w[CH * rb + c, 0, j0])
                    j1 = psi - phi - 1
                    if 0 <= j1 <= 2:
                        wd[PH * c + psi, (rb * 2 + 1) * P + PH * c + phi] = \
                            np.float16(w[CH * rb + c, 0, j1])
    bv = np.empty((P, RBLK), dtype=np.float32)
    for rb in range(RBLK):
        bv[:, rb] = np.repeat(b[CH * rb:CH * (rb + 1)], PH)
    return wd, bv


def _host_x(xj):
    # fp16 quantize + causal zero pad + phase-interleave:
    # x_il[4c+psi, m] = xpad[c, 4m+psi]
    xpad = np.zeros((C, T + PH), dtype=np.float16)
    xpad[:, PH:] = xj
    return np.ascontiguousarray(
        xpad.reshape(C, U + 1, PH).transpose(0, 2, 1).reshape(PH * C, U + 1))


def _host_y(y_il):
    # de-interleave: y[c, 4u+phi] = y_il[4c+phi, u]
    return np.ascontiguousarray(
        y_il.astype(np.float32).reshape(C, PH, U).transpose(0, 2, 1)
        .reshape(C, T))


def kernel(x, w, b):
    x = np.asarray(x, dtype=np.float32)
    w = np.asarray(w, dtype=np.float32)
    b = np.asarray(b, dtype=np.float32)

    if "nc" not in _cached:
        _cached["nc"] = _build()
    nc = _cached["nc"]

    wd, bv = _host_weights(w, b)
    in_maps = [
        {"x": _host_x(x[j]), "wd": wd, "bv": bv}
        for j in range(B)
    ]
    res = bass_utils.run_bass_kernel_spmd(nc, in_maps, core_ids=list(range(B)))
    return np.stack([_host_y(r["y"]) for r in res.results], axis=0)
